# revision 40
# baseline (speedup 1.0000x reference)
"""Correlation layer (FlowNet-style) Trainium2 Bass kernel, v2.

Problem: in1, in2: [8, 256, 128, 128] fp32.
out[b, 9*dy+dx, y, x] = mean_c in1[b,c,y,x] * in2pad[b,c,y+dy,x+dx],
in2 zero-padded by 4 per spatial side, dy,dx in [0,9).  Output
[8, 81, 128, 128] fp32.  Data-parallel over batch: 1 batch / core.

Host prep (free): in1 scaled by 1/256 (folds the channel mean),
(x-outer, y-inner) tile layout, bf16; in2 zero-padded to 136x136, bf16.
Output produced in bf16 and upcast on host.

Per-core pipeline (all bf16 matmul operands, fp32 PSUM accumulate):

1. Correlation matmuls.  Per tile xb, four col-tiled matmuls
   (tile_position (0,32g)) process the four x-columns {xb, xb+32,
   xb+64, xb+96} CONCURRENTLY on the PE array quarters: stationary =
   in1[c, 32 y-pixels of that column], moving = the column's own
   9-wide window in2pad[c, y0:y0+40, xg:xg+9] (N=360, bf16).  A tile
   costs ~2x360/2.4GHz for 128 pixels (vs 2x480 with a shared-window
   M=128 matmul), and PSUM comes out [128, 40, 9] with an identical
   free layout on every partition: pixel (g,u) holds channel ch at
   free offset 9u + ch.  The strided (not interleaved) column-to-group
   map makes each partition's scratch chunks x-consecutive, giving
   11.5 KB dump descriptors.

2. Evacuate psum -> SBUF in ONE full-width copy per tile with bf16
   cast, alternating vector/scalar.  No window-compaction needed.

3. Sheared dump (gpsimd SWDGE queue, one DRAM scratch tensor PER yb to
   avoid false whole-tensor WAR serialization between yb's dumps and
   the previous yb's xbar reads).  Chunk of pixel p' (yb-local raster
   u*128+x) starts at elem offset 360*p' - 9u: the per-partition
   shear is absorbed by the flat-DRAM u-stride (360*128-9), each
   pixel's 81 useful channel values land exactly at 360*p', and chunk
   overlaps only ever write junk over junk, so write order is free.

4. XBAR transpose read-back (sync queue): dma_start(transpose=True)
   with source AP [[360, 2048], [1, 128]] reads each pixel's 81
   channels (+47 junk cols) and transposes to SBUF [128ch, 2048pix]
   -- already in final [channel, raster(y,x)] order.

5. Store rows 0..81 to out in bf16 (scalar queue); host upcasts.
"""

import numpy as np
import ml_dtypes
from contextlib import ExitStack

import concourse.bacc as bacc
import concourse.tile as tile
import concourse.mybir as mybir
import concourse.bass as bass
from concourse import bass_utils

# ---- problem constants (hardcoded per contract) ----
B = 8
C = 256
H = W = 128
PAD = 4
D = 9            # displacements per axis
CH = D * D       # 81 output channels
HP = WP = H + 2 * PAD   # 136 padded

YB = 32          # y rows per pixel tile
XBW = 4          # x cols per pixel tile (one col-tiled matmul each)
MV_Y = YB + 8    # moving window rows per group (40)
N_YB = H // YB   # 4
N_XB = W // XBW  # 32
NG = 4           # col-tile groups per tile
TBATCH = 8       # tiles buffered per dump batch (quarter of a yb row)
NBATCH = N_XB // TBATCH  # 4

BLK = MV_Y * D   # 360 elems per pixel chunk
NPIX = H * W     # 16384
SCR_ELEMS = BLK * NPIX   # pitch-360 scratch: chunks tile exactly

XH = 4096        # pixels per xbar batch (32 y rows = one yb)
N_XBATCH = NPIX // XH  # 4

BF16 = mybir.dt.bfloat16
FP32 = mybir.dt.float32


def prep_in_maps(in1: np.ndarray, in2: np.ndarray) -> list[dict]:
    """Host-side prep: scale+layout in1, pad in2, cast bf16."""
    in1 = np.asarray(in1, dtype=np.float32)
    in2 = np.asarray(in2, dtype=np.float32)
    assert in1.shape == (B, C, H, W) and in2.shape == (B, C, H, W)
    # [B, cb, c, yb, x, y],  scaled by 1/C (folds the channel mean)
    a = (in1 * (1.0 / C)).reshape(B, 2, 128, N_YB, YB, W)
    a = np.ascontiguousarray(a.transpose(0, 1, 2, 3, 5, 4)).astype(
        ml_dtypes.bfloat16
    )
    # [B, cb, c, 128, 136] x-padded only (y-pad rows are memset on SBUF)
    p = np.pad(in2, ((0, 0), (0, 0), (0, 0), (PAD, PAD))).reshape(
        B, 2, 128, H, WP
    ).astype(ml_dtypes.bfloat16)
    return [{"in1": a[b], "in2": p[b]} for b in range(B)]


def build_nc():
    nc = bacc.Bacc("TRN2", target_bir_lowering=False, debug=False)
    in1_d = nc.dram_tensor(
        "in1", [2, 128, N_YB, W, YB], BF16, kind="ExternalInput"
    ).ap()
    in2_d = nc.dram_tensor(
        "in2", [2, 128, H, WP], BF16, kind="ExternalInput"
    ).ap()
    out_d = nc.dram_tensor("out", [CH, H, W], BF16, kind="ExternalOutput").ap()
    # one scratch tensor per yb: DRAM deps are tracked per-tensor, so a
    # shared scratch would serialize yb+1's dumps behind yb's xbar reads
    scr_t = [
        nc.dram_tensor(f"scr{yb}", [SCR_ELEMS // N_YB], BF16, kind="Internal")
        for yb in range(N_YB)
    ]

    with tile.TileContext(nc) as tc, ExitStack() as es:
        in2_pool = es.enter_context(tc.tile_pool(name="in2p", bufs=1))
        in1_pool = es.enter_context(tc.tile_pool(name="in1c", bufs=1))
        wv_pool = es.enter_context(tc.tile_pool(name="wv", bufs=6))
        xb_pool = es.enter_context(tc.tile_pool(name="xb", bufs=2))
        psum_pool = es.enter_context(tc.tile_pool(name="ps", bufs=8, space="PSUM"))

        # ---- inputs split over both HWDGE queues (in1 on sync, in2 on
        # scalar) in first-use order: the first matmul waits only for
        # ~2.4 MB and the two streams load in parallel. ----
        in2p = in2_pool.tile([128, 2, HP, WP], BF16, tag="in2p")
        in1c = in1_pool.tile([128, 2, N_YB, W, YB], BF16, tag="in1c")
        # y-pad rows are zeroed on-chip; loads carry interior rows only
        nc.vector.memset(in2p[:, :, 0:PAD, :], 0.0)
        nc.gpsimd.memset(in2p[:, :, HP - PAD:HP, :], 0.0)
        row_chunks = [(PAD, 40), (40, 72), (72, 104), (104, HP - PAD)]
        for yb in range(N_YB):
            r0, r1 = row_chunks[yb]
            for cb in range(2):
                nc.sync.dma_start(
                    in1c[:, cb, yb, :, :], in1_d[cb, :, yb, :, :]
                )
                nc.scalar.dma_start(
                    in2p[:, cb, r0:r1, :], in2_d[cb, :, r0 - PAD:r1 - PAD, :]
                )

        for yb in range(N_YB):
            y0 = yb * YB
            for qtr in range(NBATCH):
                xbase = qtr * TBATCH
                wv = wv_pool.tile([128, TBATCH, MV_Y, D], BF16, tag="wv")
                for t in range(TBATCH):
                    xb = xbase + t
                    ps = psum_pool.tile([128, MV_Y, D], FP32, tag="ps")
                    for cb in range(2):
                        for g in range(NG):
                            # group g = x-column 32g + xb (strided!):
                            # partition 32g+u <-> pixel (x=32g+xb, y)
                            # so each partition's scratch chunks are
                            # x-consecutive -> 11.5 KB dump descriptors
                            xg = 32 * g + xb
                            stat = in1c[:, cb, yb, xg, :]
                            mov = in2p[:, cb, y0:y0 + MV_Y, xg:xg + D]
                            nc.tensor.matmul(
                                ps[32 * g:32 * (g + 1), :, :],
                                stat,
                                mov,
                                start=(cb == 0),
                                stop=(cb == 1),
                                tile_position=(0, 32 * g),
                            )
                    dst = wv[:, t, :, :]
                    if xb % 2 == 0:
                        nc.vector.tensor_copy(dst, ps[:, :, :])
                    else:
                        nc.scalar.copy(dst, ps[:, :, :])

                # sheared dump (gpsimd SWDGE queue; scalar/sync would
                # let the dump's in-queue evac-wait block evacs/xbars):
                # chunk of pixel p' (yb-local raster) at 360*p' - 9u;
                # useful 81 land at 360*p'.  Chunk overlaps are
                # junk-over-junk -> order-free.
                for g in range(NG):
                    src = wv[32 * g:32 * (g + 1), :, :, :].rearrange(
                        "p t a b -> p (t a b)"
                    )
                    base = BLK * (32 * g + xbase)
                    dst = bass.AP(
                        scr_t[yb],
                        base,
                        [[BLK * W - D, 32], [1, BLK * TBATCH]],
                    )
                    nc.gpsimd.dma_start(dst, src)

            # ---- read back via one whole-yb xbar transpose + store on
            # the sync queue.  Keeping stores off the scalar queue
            # matters: a store's in-queue wait for its xbar would
            # otherwise block the next yb's scalar evacs.
            xbt = xb_pool.tile([128, XH], BF16, tag="xbt")
            src = bass.AP(scr_t[yb], 0, [[BLK, XH], [1, 128]])
            nc.sync.dma_start(xbt[:, :], src, transpose=True)
            store = out_d[:, YB * yb:YB * (yb + 1), :].rearrange(
                "c a b -> c (a b)"
            )
            nc.sync.dma_start(store, xbt[0:CH, :])

    nc.compile()
    return nc


_NC_CACHE = None


def _get_nc():
    global _NC_CACHE
    if _NC_CACHE is None:
        _NC_CACHE = build_nc()
    return _NC_CACHE


def kernel(in1: np.ndarray, in2: np.ndarray) -> np.ndarray:
    nc = _get_nc()
    in_maps = prep_in_maps(in1, in2)
    res = bass_utils.run_bass_kernel_spmd(nc, in_maps, core_ids=list(range(B)))
    return np.stack(
        [res.results[b]["out"].astype(np.float32) for b in range(B)], axis=0
    )


# revision 41
# speedup vs baseline: 1.0280x; 1.0280x over previous
"""Correlation layer (FlowNet-style) Trainium2 Bass kernel, v2.

Problem: in1, in2: [8, 256, 128, 128] fp32.
out[b, 9*dy+dx, y, x] = mean_c in1[b,c,y,x] * in2pad[b,c,y+dy,x+dx],
in2 zero-padded by 4 per spatial side, dy,dx in [0,9).  Output
[8, 81, 128, 128] fp32.  Data-parallel over batch: 1 batch / core.

Host prep (free): in1 scaled by 1/256 (folds the channel mean),
(x-outer, y-inner) tile layout, bf16; in2 zero-padded to 136x136, bf16.
Output produced in bf16 and upcast on host.

Per-core pipeline (all bf16 matmul operands, fp32 PSUM accumulate):

1. Correlation matmuls.  Per tile xb, four col-tiled matmuls
   (tile_position (0,32g)) process the four x-columns {xb, xb+32,
   xb+64, xb+96} CONCURRENTLY on the PE array quarters: stationary =
   in1[c, 32 y-pixels of that column], moving = the column's own
   9-wide window in2pad[c, y0:y0+40, xg:xg+9] (N=360, bf16).  A tile
   costs ~2x360/2.4GHz for 128 pixels (vs 2x480 with a shared-window
   M=128 matmul), and PSUM comes out [128, 40, 9] with an identical
   free layout on every partition: pixel (g,u) holds channel ch at
   free offset 9u + ch.  The strided (not interleaved) column-to-group
   map makes each partition's scratch chunks x-consecutive, giving
   11.5 KB dump descriptors.

2. Evacuate psum -> SBUF in ONE full-width copy per tile with bf16
   cast, alternating vector/scalar.  No window-compaction needed.

3. Sheared dump (gpsimd SWDGE queue, one DRAM scratch tensor PER yb to
   avoid false whole-tensor WAR serialization between yb's dumps and
   the previous yb's xbar reads).  Chunk of pixel p' (yb-local raster
   u*128+x) starts at elem offset 360*p' - 9u: the per-partition
   shear is absorbed by the flat-DRAM u-stride (360*128-9), each
   pixel's 81 useful channel values land exactly at 360*p', and chunk
   overlaps only ever write junk over junk, so write order is free.

4. XBAR transpose read-back (sync queue): dma_start(transpose=True)
   with source AP [[360, 2048], [1, 128]] reads each pixel's 81
   channels (+47 junk cols) and transposes to SBUF [128ch, 2048pix]
   -- already in final [channel, raster(y,x)] order.

5. Store rows 0..81 to out in bf16 (scalar queue); host upcasts.
"""

import numpy as np
import ml_dtypes
from contextlib import ExitStack

import concourse.bacc as bacc
import concourse.tile as tile
import concourse.mybir as mybir
import concourse.bass as bass
from concourse import bass_utils

# ---- problem constants (hardcoded per contract) ----
B = 8
C = 256
H = W = 128
PAD = 4
D = 9            # displacements per axis
CH = D * D       # 81 output channels
HP = WP = H + 2 * PAD   # 136 padded

YB = 32          # y rows per pixel tile
XBW = 4          # x cols per pixel tile (one col-tiled matmul each)
MV_Y = YB + 8    # moving window rows per group (40)
N_YB = H // YB   # 4
N_XB = W // XBW  # 32
NG = 4           # col-tile groups per tile
TBATCH = 8       # tiles buffered per dump batch (quarter of a yb row)
NBATCH = N_XB // TBATCH  # 4

BLK = MV_Y * D   # 360 elems per pixel chunk
NPIX = H * W     # 16384
SCR_ELEMS = BLK * NPIX   # pitch-360 scratch: chunks tile exactly

XH = 4096        # pixels per xbar batch (32 y rows = one yb)
N_XBATCH = NPIX // XH  # 4

BF16 = mybir.dt.bfloat16
FP32 = mybir.dt.float32


def prep_in_maps(in1: np.ndarray, in2: np.ndarray) -> list[dict]:
    """Host-side prep: scale+layout in1, pad in2, cast bf16."""
    in1 = np.asarray(in1, dtype=np.float32)
    in2 = np.asarray(in2, dtype=np.float32)
    assert in1.shape == (B, C, H, W) and in2.shape == (B, C, H, W)
    # [B, cb, c, yb, x, y],  scaled by 1/C (folds the channel mean)
    a = (in1 * (1.0 / C)).reshape(B, 2, 128, N_YB, YB, W)
    a = np.ascontiguousarray(a.transpose(0, 1, 2, 3, 5, 4)).astype(
        ml_dtypes.bfloat16
    )
    # [B, cb, c, 128, 136] x-padded only (y-pad rows are memset on SBUF)
    p = np.pad(in2, ((0, 0), (0, 0), (0, 0), (PAD, PAD))).reshape(
        B, 2, 128, H, WP
    ).astype(ml_dtypes.bfloat16)
    return [{"in1": a[b], "in2": p[b]} for b in range(B)]


def build_nc():
    nc = bacc.Bacc("TRN2", target_bir_lowering=False, debug=False)
    in1_d = nc.dram_tensor(
        "in1", [2, 128, N_YB, W, YB], BF16, kind="ExternalInput"
    ).ap()
    in2_d = nc.dram_tensor(
        "in2", [2, 128, H, WP], BF16, kind="ExternalInput"
    ).ap()
    out_d = nc.dram_tensor("out", [CH, H, W], BF16, kind="ExternalOutput").ap()
    # one scratch tensor per yb: DRAM deps are tracked per-tensor, so a
    # shared scratch would serialize yb+1's dumps behind yb's xbar reads
    scr_t = [
        nc.dram_tensor(f"scr{yb}", [SCR_ELEMS // N_YB], BF16, kind="Internal")
        for yb in range(N_YB)
    ]

    with tile.TileContext(nc) as tc, ExitStack() as es:
        in2_pool = es.enter_context(tc.tile_pool(name="in2p", bufs=1))
        in1_pool = es.enter_context(tc.tile_pool(name="in1c", bufs=1))
        wv_pool = es.enter_context(tc.tile_pool(name="wv", bufs=8))
        xb_pool = es.enter_context(tc.tile_pool(name="xb", bufs=2))
        psum_pool = es.enter_context(tc.tile_pool(name="ps", bufs=8, space="PSUM"))

        # ---- inputs split over both HWDGE queues (in1 on sync, in2 on
        # scalar) in first-use order: the first matmul waits only for
        # ~2.4 MB and the two streams load in parallel. ----
        in2p = in2_pool.tile([128, 2, HP, WP], BF16, tag="in2p")
        in1c = in1_pool.tile([128, 2, N_YB, W, YB], BF16, tag="in1c")
        # y-pad rows are zeroed on-chip; loads carry interior rows only
        nc.vector.memset(in2p[:, :, 0:PAD, :], 0.0)
        nc.gpsimd.memset(in2p[:, :, HP - PAD:HP, :], 0.0)
        row_chunks = [(PAD, 40), (40, 72), (72, 104), (104, HP - PAD)]
        for yb in range(N_YB):
            r0, r1 = row_chunks[yb]
            for cb in range(2):
                nc.sync.dma_start(
                    in1c[:, cb, yb, :, :], in1_d[cb, :, yb, :, :]
                )
                nc.scalar.dma_start(
                    in2p[:, cb, r0:r1, :], in2_d[cb, :, r0 - PAD:r1 - PAD, :]
                )

        for yb in range(N_YB):
            y0 = yb * YB
            for qtr in range(NBATCH):
                xbase = qtr * TBATCH
                wv = wv_pool.tile([128, TBATCH, MV_Y, D], BF16, tag="wv")
                for t in range(TBATCH):
                    xb = xbase + t
                    ps = psum_pool.tile([128, MV_Y, D], FP32, tag="ps")
                    for cb in range(2):
                        for g in range(NG):
                            # group g = x-column 32g + xb (strided!):
                            # partition 32g+u <-> pixel (x=32g+xb, y)
                            # so each partition's scratch chunks are
                            # x-consecutive -> 11.5 KB dump descriptors
                            xg = 32 * g + xb
                            stat = in1c[:, cb, yb, xg, :]
                            mov = in2p[:, cb, y0:y0 + MV_Y, xg:xg + D]
                            nc.tensor.matmul(
                                ps[32 * g:32 * (g + 1), :, :],
                                stat,
                                mov,
                                start=(cb == 0),
                                stop=(cb == 1),
                                tile_position=(0, 32 * g),
                            )
                    dst = wv[:, t, :, :]
                    if xb % 2 == 0:
                        nc.vector.tensor_copy(dst, ps[:, :, :])
                    else:
                        nc.scalar.copy(dst, ps[:, :, :])

                # sheared dump (gpsimd SWDGE queue; scalar/sync would
                # let the dump's in-queue evac-wait block evacs/xbars):
                # chunk of pixel p' (yb-local raster) at 360*p' - 9u;
                # useful 81 land at 360*p'.  Chunk overlaps are
                # junk-over-junk -> order-free.
                for g in range(NG):
                    src = wv[32 * g:32 * (g + 1), :, :, :].rearrange(
                        "p t a b -> p (t a b)"
                    )
                    base = BLK * (32 * g + xbase)
                    dst = bass.AP(
                        scr_t[yb],
                        base,
                        [[BLK * W - D, 32], [1, BLK * TBATCH]],
                    )
                    nc.gpsimd.dma_start(dst, src)

            # ---- read back via one whole-yb xbar transpose + store on
            # the sync queue.  Keeping stores off the scalar queue
            # matters: a store's in-queue wait for its xbar would
            # otherwise block the next yb's scalar evacs.
            xbt = xb_pool.tile([128, XH], BF16, tag="xbt")
            src = bass.AP(scr_t[yb], 0, [[BLK, XH], [1, 128]])
            nc.sync.dma_start(xbt[:, :], src, transpose=True)
            store = out_d[:, YB * yb:YB * (yb + 1), :].rearrange(
                "c a b -> c (a b)"
            )
            nc.sync.dma_start(store, xbt[0:CH, :])

    nc.compile()
    return nc


_NC_CACHE = None


def _get_nc():
    global _NC_CACHE
    if _NC_CACHE is None:
        _NC_CACHE = build_nc()
    return _NC_CACHE


def kernel(in1: np.ndarray, in2: np.ndarray) -> np.ndarray:
    nc = _get_nc()
    in_maps = prep_in_maps(in1, in2)
    res = bass_utils.run_bass_kernel_spmd(nc, in_maps, core_ids=list(range(B)))
    return np.stack(
        [res.results[b]["out"].astype(np.float32) for b in range(B)], axis=0
    )
